# revision 21
# baseline (speedup 1.0000x reference)
"""CosFormer causal attention — Trainium2 Bass kernel, 8 NeuronCores.

Sharding: core i = (batch b = i//4, head-group g = i%4 covering heads 2g, 2g+1).
Each core computes the qkv projection for its two heads, chunked causal linear
attention (cos/sin feature channels), and a partial output projection over its
128 context channels. The host unshards by summing the 4 per-core partials of
each batch (the output projection's contraction is sharded over heads) and
adding b_out.

v2 perf changes vs baseline:
- fp16 everywhere on the PE (1 cyc/row vs ~2 for fp32r, FWL weight loads,
  half the DMA bytes). PSUM stays fp32; fp16 out partials (host sums in f32).
- QKV: block-outer / kk-inner / t-half-paired so each LDWEIGHTS feeds two
  512-wide matmuls.
- Normalization moved BEFORE the output projection (1/norm broadcast across
  partitions via a K=1 ones matmul), letting both heads share one full-128-
  contraction out-projection matmul per 128 queries (half the out-proj work,
  and the per-head [128,512] scale/add epilogues disappear).
- 8 identity warm-up matmuls at t=0 keep the PE HAM busy while input DMA
  streams; DMA issue order = exactly the consumption order.
"""

import math
from contextlib import ExitStack

import numpy as np

import concourse.bass as bass
import concourse.mybir as mybir
import concourse.tile as tile
from concourse.bass_utils import run_bass_kernel_spmd
from concourse.vector_clock import ScopedClock

B, T, E = 2, 1024, 512
H, D = 8, 64
S = 128            # key stripe size
SC = 256           # query super-chunk size
NSC = T // SC      # 4
F32 = mybir.dt.float32
F32R = mybir.dt.float32r
F16 = mybir.dt.float16
EPS = 1e-6
NWARM = 8


def _install_drain_patch():
    """This walrus build rejects a Drain carrying >1 sem wait. Split the
    Tile-exit drain's waits across single-wait SP nops."""
    if getattr(tile.TileContext, "_drain_patch_installed", False):
        return

    def _patched(self, tick_clock, wait_clock):
        nc = self.nc
        pre = nc.sync.nop(nofuse=True)
        wait_clock.add_sem_waits(pre.ins, ScopedClock({None: tick_clock.global_clock}))
        waits = list(pre.ins.sync_info.on_wait or []) if pre.ins.sync_info else []
        if len(waits) > 1:
            pre.ins.sync_info.on_wait = waits[:1]
            for w in waits[1:]:
                n = nc.sync.nop(nofuse=True)
                if n.ins.sync_info is None:
                    n.ins.sync_info = mybir.SyncInfo(on_wait=[w], on_update=[])
                else:
                    n.ins.sync_info.on_wait = [w]
        nc.sync.drain()
        nc.all_engine_barrier()
        popped = nc._tile_sem_poison_stack.pop()
        assert popped is self._sem_poison

    tile.TileContext._drain_and_barrier = _patched
    tile.TileContext._drain_patch_installed = True


def _split_multi_waits(nc):
    """This walrus build only codegens ONE sync-wait command per instruction.
    Move excess waits onto same-engine NoOps inserted just before."""
    ctr = [0]

    def _mk_nop(engine, wait):
        ctr[0] += 1
        return mybir.InstNoOp(
            name=f"I-waitnop{ctr[0]}",
            engine=engine,
            ins=[],
            outs=[],
            sync_info=mybir.SyncInfo(on_wait=[wait], on_update=[]),
        )

    for f in nc.m.functions:
        for bb in f.blocks:
            new_insts = []
            for inst in bb.instructions:
                si = inst.sync_info
                waits = list(si.on_wait) if si and si.on_wait else []
                if len(waits) > 1:
                    for w in waits[:-1]:
                        new_insts.append(_mk_nop(inst.engine, w))
                    si.on_wait = waits[-1:]
                new_insts.append(inst)
            bb.instructions[:] = new_insts


def build_program() -> bass.Bass:
    _install_drain_patch()
    nc = bass.Bass()

    # wqkf: duplicated weight cols [qf_h0 | qf_h1 | kf_h0 | kf_h1], each 128 wide
    xt = nc.declare_dram_parameter("xt", [E, T], F16, isOutput=False)        # x[b].T
    wqkf = nc.declare_dram_parameter("wqkf", [E, 512], F16, isOutput=False)
    wvt = nc.declare_dram_parameter("wvt", [E, 128], F16, isOutput=False)    # [v0 v1].T
    ball = nc.declare_dram_parameter("ball", [640], F32, isOutput=False)     # dup'd qk biases + v bias
    csrep = nc.declare_dram_parameter("csrep", [128, T], F16, isOutput=False)  # [cos;sin]
    w2 = nc.declare_dram_parameter("w2", [128, E], F16, isOutput=False)
    identin = nc.declare_dram_parameter("identin", [128, 128], F16, isOutput=False)
    m0in = nc.declare_dram_parameter("m0in", [S, SC], F16, isOutput=False)   # [tri | ones]
    out = nc.declare_dram_parameter("out", [T, E], F16, isOutput=True)
    out_r = out.rearrange("(s ci p) e -> p s ci e", ci=2, p=128)

    with tile.TileContext(nc) as tc, ExitStack() as ctx:
        singles = ctx.enter_context(tc.tile_pool(name="singles", bufs=1))
        kf_pool = ctx.enter_context(tc.tile_pool(name="kf", bufs=4))
        atm_pool = ctx.enter_context(tc.tile_pool(name="atm", bufs=3))
        osb_pool = ctx.enter_context(tc.tile_pool(name="osb", bufs=2))
        nrm_pool = ctx.enter_context(tc.tile_pool(name="nrm", bufs=4))
        ctx_pool = ctx.enter_context(tc.tile_pool(name="ctxp", bufs=2))
        pp_big = ctx.enter_context(tc.tile_pool(name="pp_big", bufs=2, space="PSUM"))
        pp_mm = ctx.enter_context(tc.tile_pool(name="pp_mm", bufs=2, space="PSUM"))
        pp_kt = ctx.enter_context(tc.tile_pool(name="pp_kt", bufs=1, space="PSUM"))
        pp_cs = ctx.enter_context(tc.tile_pool(name="pp_cs", bufs=3, space="PSUM"))

        # ---- input DMAs, exactly in consumption order on the sync queue ---
        ident = singles.tile([128, 128], F16)
        nc.sync.dma_start(out=ident, in_=identin[:, :])
        xt_s = singles.tile([128, 4, T], F16)
        xt_r = xt.rearrange("(kk p) t -> p kk t", p=128)
        wqkf_s = singles.tile([128, 4, 512], F16)
        wqkf_r = wqkf.rearrange("(kk p) c -> p kk c", p=128)
        biasall = singles.tile([128, 5], F32, name="biasall")
        cs_s = singles.tile([128, T], F16)
        for kk in range(4):
            nc.sync.dma_start(out=wqkf_s[:, kk, :], in_=wqkf_r[:, kk, :])
            nc.sync.dma_start(out=xt_s[:, kk, :], in_=xt_r[:, kk, :])
            if kk == 0:
                nc.sync.dma_start(out=biasall, in_=ball.rearrange("(g p) -> p g", p=128))
                nc.sync.dma_start(out=cs_s, in_=csrep[:, :])
        wvt_s = singles.tile([128, 4, 128], F16)
        nc.sync.dma_start(out=wvt_s, in_=wvt.rearrange("(kk p) c -> p kk c", p=128))
        m0_s = singles.tile([S, SC], F16)
        nc.sync.dma_start(out=m0_s, in_=m0in[:, :])
        w2_s = singles.tile([128, E], F16, name="w2_s")
        nc.sync.dma_start(out=w2_s, in_=w2[:, :])

        ones16 = singles.tile([1, 64], F16, name="ones16")
        nc.vector.memset(ones16, 1.0)
        onesz_col = singles.tile([128, 2], F16, name="onesz_col")
        nc.vector.memset(onesz_col[:, 0:1], 1.0)
        nc.vector.memset(onesz_col[:, 1:2], 0.0)

        # ---- PE warm-up on a zero scratch tile: starts as soon as the
        # vector engine is up, needs NO input DMA, keeps the HAM clock-gate
        # busy so real matmuls run at 2.4 GHz.
        junk = singles.tile([128, 512], F16, name="junk")
        nc.vector.memset(junk, 0.0)
        for wi in range(NWARM):
            psw = pp_big.tile([128, 512], F32, tag="big", name=f"psw{wi}")
            nc.tensor.matmul(psw, junk[:, 0:128], junk, start=True, stop=True)

        # per-head stacked feature tiles [cos;sin] x t
        qfT = [singles.tile([128, T], F16, name=f"qfT{h}") for h in range(2)]
        kfT = [singles.tile([128, T], F16, name=f"kfT{h}") for h in range(2)]
        vT = singles.tile([128, T], F16, name="vT")
        state = [singles.tile([128, D + 2], F16, name=f"state{h}") for h in range(2)]
        # persistent V' ring: [head][stripe], ones/pad cols written once
        vp_ring = [[singles.tile([S, D + 2], F16, name=f"vpr{h}_{ci}")
                    for ci in range(2)] for h in range(2)]
        for h in range(2):
            for ci in range(2):
                nc.scalar.copy(vp_ring[h][ci][:, D:D + 2], onesz_col)

        # ---- q/k/v features: block-outer, kk-inner, t-halves paired ------
        # block bi: 0=qf_h0, 1=qf_h1, 2=kf_h0, 3=kf_h1, 4=v
        for bi in range(5):
            ps_h = [pp_big.tile([128, 512], F32, tag="big", name=f"psB{bi}_{th}")
                    for th in range(2)]
            for kk in range(4):
                w = wqkf_s[:, kk, bi * 128:(bi + 1) * 128] if bi < 4 else wvt_s[:, kk, :]
                for th in range(2):
                    nc.tensor.matmul(ps_h[th], w, xt_s[:, kk, th * 512:(th + 1) * 512],
                                     start=(kk == 0), stop=(kk == 3))
            for th in range(2):
                tslh = slice(th * 512, (th + 1) * 512)
                if bi < 4:
                    dst = [qfT[0], qfT[1], kfT[0], kfT[1]][bi]
                    nc.scalar.activation(
                        out=dst[:, tslh],
                        in_=ps_h[th],
                        func=mybir.ActivationFunctionType.Relu,
                        bias=biasall[:, bi:bi + 1],
                        scale=1.0,
                    )
                    nc.vector.tensor_mul(dst[:, tslh], dst[:, tslh], cs_s[:, tslh])
                else:
                    nc.scalar.activation(
                        out=vT[:, tslh],
                        in_=ps_h[th],
                        func=mybir.ActivationFunctionType.Identity,
                        bias=biasall[:, 4:5],
                        scale=1.0,
                    )

        # ---- attention, 256-wide query super-chunks, software-pipelined:
        # super-chunk sc's matmuls are emitted BEFORE sc-1's norm/out-proj
        # tail so the PE queue never stalls on the reciprocal chain.
        def emit_tail(psc2, sc):
            nrow = nrm_pool.tile([1, 2 * SC], F32, tag="nrow", name=f"nrow{sc}")
            for h in range(2):
                nc.scalar.copy(nrow[0:1, h * SC:(h + 1) * SC], psc2[D:D + 1, h, :])
            rcpf = nrm_pool.tile([1, 2 * SC], F32, tag="rcpf", name=f"rcpf{sc}")
            nc.vector.reciprocal(rcpf, nrow)
            rcp16 = nrm_pool.tile([1, 2 * SC], F16, tag="rcp16", name=f"rcp16_{sc}")
            for h in range(2):
                nc.gpsimd.tensor_copy(rcp16[0:1, h * SC:(h + 1) * SC],
                                      rcpf[0:1, h * SC:(h + 1) * SC])
            ps_rcp = pp_mm.tile([D, 2 * SC], F32, tag="mm", name=f"psrcp{sc}")
            nc.tensor.matmul(ps_rcp, ones16, rcp16, start=True, stop=True)
            rcp2 = nrm_pool.tile([D, 2 * SC], F16, tag="rcp2", name=f"rcp2{sc}")
            nc.scalar.copy(rcp2[:, 0:SC], ps_rcp[:, 0:SC])
            nc.vector.tensor_copy(rcp2[:, SC:2 * SC], ps_rcp[:, SC:2 * SC])

            # normalized two-head context block [2d, t], then one combined
            # out-projection per 128 queries (full 128 contraction)
            ctx2 = ctx_pool.tile([128, SC], F16, tag="ctx2", name=f"ctx2{sc}")
            nc.vector.tensor_mul(ctx2[0:D, :], psc2[0:D, 0, :], rcp2[:, 0:SC])
            nc.vector.tensor_mul(ctx2[D:2 * D, :], psc2[0:D, 1, :], rcp2[:, SC:2 * SC])

            o_s2 = osb_pool.tile([128, 2, E], F16, tag="osb", name=f"os{sc}")
            for ci in range(2):
                ps_o = pp_big.tile([128, E], F32, tag="big", name=f"pso{sc}_{ci}")
                nc.tensor.matmul(ps_o, ctx2[:, ci * S:(ci + 1) * S], w2_s,
                                 start=True, stop=True)
                if ci == 0:
                    nc.vector.tensor_copy(o_s2[:, ci, :], ps_o)
                else:
                    nc.scalar.copy(o_s2[:, ci, :], ps_o)
            nc.sync.dma_start(out=out_r[:, sc, :, :], in_=o_s2)

        pipe = []  # (psc2, sc) tuples awaiting their norm/out-proj tails
        for sc in range(NSC):
            t0 = sc * SC
            band = slice(t0, t0 + SC)
            sub = [slice(t0, t0 + S), slice(t0 + S, t0 + 2 * S)]

            # stripe transposes: kfT/vT [*, t] -> [t, *] per 128-stripe
            kfeat = [[None, None], [None, None]]  # [ci][h]
            vp = [[None, None], [None, None]]     # [ci][h]
            for ci in range(2):
                ps_kt = pp_kt.tile([128, 384], F16, tag="kt", name=f"pskt{sc}_{ci}")
                for h in range(2):
                    kfeat[ci][h] = kf_pool.tile(
                        [S, 128], F16, tag=f"kf{h}", name=f"kfeat{sc}_{ci}_{h}")
                    nc.tensor.transpose(
                        ps_kt[:, h * 128:(h + 1) * 128], kfT[h][:, sub[ci]], ident)
                nc.vector.tensor_copy(kfeat[ci][0], ps_kt[:, 0:128])
                nc.scalar.copy(kfeat[ci][1], ps_kt[:, 128:256])
                nc.tensor.transpose(ps_kt[:, 256:384], vT[:, sub[ci]], ident)
                for h in range(2):
                    vp[ci][h] = vp_ring[h][ci]
                nc.vector.tensor_copy(vp[ci][0][:, 0:D], ps_kt[:, 256:256 + D])
                nc.scalar.copy(vp[ci][1][:, 0:D], ps_kt[:, 256 + D:256 + 2 * D])

            # both heads' ctx^T (+norm row 64) share one PSUM bank so two
            # super-chunks can be in flight (pipelined tail)
            psc2 = pp_cs.tile([D + 2, 2, SC], F32, tag="cs", name=f"psc{sc}")
            for h in range(2):
                # stripe 0 scores the whole band; stripe 1 only its own half
                # (both score blocks share one bank: one accumulation group)
                psa = pp_mm.tile([S, SC + S], F32, tag="mm", name=f"psa{sc}_{h}")
                nc.tensor.matmul(psa[:, 0:SC], kfT[h][:, sub[0]], qfT[h][:, band],
                                 start=True, stop=False)
                nc.tensor.matmul(psa[:, SC:SC + S], kfT[h][:, sub[1]], qfT[h][:, sub[1]],
                                 start=False, stop=True)
                atm0 = atm_pool.tile([S, SC], F16, tag="atm", name=f"atm{sc}_0_{h}")
                nc.vector.tensor_mul(atm0, psa[:, 0:SC], m0_s)
                atm1 = atm_pool.tile([S, S], F16, tag="atm1", name=f"atm{sc}_1_{h}")
                nc.vector.tensor_mul(atm1, psa[:, SC:SC + S], m0_s[:, 0:S])

                # ctx^T: prefix-state inter + two stripe intras; the two heads
                # form ONE accumulation group (h0 starts/clears the bank, h1's
                # fresh-region writes land via has_written=0)
                st = (h == 0)
                if sc > 0:
                    nc.tensor.matmul(psc2[:, h, :], state[h], qfT[h][:, band],
                                     start=st, stop=False)
                    nc.tensor.matmul(psc2[:, h, :], vp[0][h], atm0,
                                     start=False, stop=False)
                    nc.tensor.matmul(psc2[:, h, S:SC], vp[1][h], atm1,
                                     start=False, stop=(h == 1))
                else:
                    nc.tensor.matmul(psc2[:, h, :], vp[0][h], atm0,
                                     start=st, stop=False)
                    nc.tensor.matmul(psc2[:, h, S:SC], vp[1][h], atm1,
                                     start=False, stop=(h == 1))

                # state += Kf^T V' over both stripes
                ps_s = pp_mm.tile([128, D + 2], F32, tag="mm", name=f"pss{sc}_{h}")
                nc.tensor.matmul(ps_s, kfeat[0][h], vp[0][h], start=True, stop=False)
                nc.tensor.matmul(ps_s, kfeat[1][h], vp[1][h], start=False, stop=True)
                if sc == 0:
                    nc.vector.tensor_copy(state[h], ps_s)
                else:
                    nc.vector.tensor_add(state[h], state[h], ps_s)

            pipe.append((psc2, sc))
            if len(pipe) > 2:
                emit_tail(*pipe.pop(0))
        for p in pipe:
            emit_tail(*p)

    _split_multi_waits(nc)
    return nc


_PROGRAM = None


def _get_program():
    global _PROGRAM
    if _PROGRAM is None:
        _PROGRAM = build_program()
    return _PROGRAM


def _make_in_maps(x, w_qkv, b_qkv, w_out):
    pos = np.arange(T, dtype=np.float32)
    ang = (math.pi / 2) * pos / T
    cosw = np.cos(ang).astype(np.float32)
    sinw = np.sin(ang).astype(np.float32)
    csrep = np.concatenate([
        np.broadcast_to(cosw[None, :], (D, T)),
        np.broadcast_to(sinw[None, :], (D, T)),
    ], 0).astype(np.float16)
    tri = np.triu(np.ones((S, S), np.float16))
    m0 = np.concatenate([tri, np.ones((S, S), np.float16)], 1)

    in_maps = []
    for i in range(8):
        b, g = divmod(i, 4)
        h0, h1 = 2 * g, 2 * g + 1
        wq = lambda h: w_qkv[h * D:(h + 1) * D]
        wk = lambda h: w_qkv[E + h * D:E + (h + 1) * D]
        wv = lambda h: w_qkv[2 * E + h * D:2 * E + (h + 1) * D]
        bq = lambda h: b_qkv[h * D:(h + 1) * D]
        bk = lambda h: b_qkv[E + h * D:E + (h + 1) * D]
        bv = lambda h: b_qkv[2 * E + h * D:2 * E + (h + 1) * D]
        hcols = np.r_[h0 * D:(h0 + 1) * D, h1 * D:(h1 + 1) * D]
        wqkf = np.concatenate([
            wq(h0), wq(h0), wq(h1), wq(h1), wk(h0), wk(h0), wk(h1), wk(h1)
        ], 0).T
        ball = np.concatenate([
            bq(h0), bq(h0), bq(h1), bq(h1), bk(h0), bk(h0), bk(h1), bk(h1),
            bv(h0), bv(h1)
        ]).astype(np.float32)
        in_maps.append({
            "xt": np.ascontiguousarray(x[b].T).astype(np.float16),
            "wqkf": np.ascontiguousarray(wqkf).astype(np.float16),
            "wvt": np.ascontiguousarray(
                np.concatenate([wv(h0), wv(h1)], 0).T).astype(np.float16),
            "ball": np.ascontiguousarray(ball),
            "csrep": csrep,
            "w2": np.ascontiguousarray(w_out[:, hcols].T).astype(np.float16),
            "identin": np.eye(128, dtype=np.float16),
            "m0in": m0,
        })
    return in_maps


def run(inputs, trace=False):
    x = np.asarray(inputs["x"], dtype=np.float32)
    w_qkv = np.asarray(inputs["w_qkv"], dtype=np.float32)
    b_qkv = np.asarray(inputs["b_qkv"], dtype=np.float32)
    w_out = np.asarray(inputs["w_out"], dtype=np.float32)
    b_out = np.asarray(inputs["b_out"], dtype=np.float32)

    nc = _get_program()
    in_maps = _make_in_maps(x, w_qkv, b_qkv, w_out)
    res = run_bass_kernel_spmd(nc, in_maps, list(range(8)), trace=trace)

    out = np.empty((B, T, E), dtype=np.float32)
    for b in range(B):
        acc = res.results[4 * b]["out"].astype(np.float32)
        for g in range(1, 4):
            acc = acc + res.results[4 * b + g]["out"].astype(np.float32)
        out[b] = acc + b_out[None, :]
    return out, res


def kernel(**inputs) -> np.ndarray:
    out, _ = run(inputs, trace=False)
    return out


# revision 28
# speedup vs baseline: 1.1023x; 1.1023x over previous
"""CosFormer causal attention — Trainium2 Bass kernel, 8 NeuronCores.

Sharding: core i = (batch b = i//4, head-group g = i%4 covering heads 2g, 2g+1).
Each core computes the qkv projection for its two heads, chunked causal linear
attention (cos/sin feature channels), and a partial output projection over its
128 context channels. The host unshards by summing the 4 per-core partials of
each batch (the output projection's contraction is sharded over heads) and
adding b_out.

v2 perf changes vs baseline:
- fp16 everywhere on the PE (1 cyc/row vs ~2 for fp32r, FWL weight loads,
  half the DMA bytes). PSUM stays fp32; fp16 out partials (host sums in f32).
- QKV: block-outer / kk-inner / t-half-paired so each LDWEIGHTS feeds two
  512-wide matmuls.
- Normalization moved BEFORE the output projection (1/norm broadcast across
  partitions via a K=1 ones matmul), letting both heads share one full-128-
  contraction out-projection matmul per 128 queries (half the out-proj work,
  and the per-head [128,512] scale/add epilogues disappear).
- 8 identity warm-up matmuls at t=0 keep the PE HAM busy while input DMA
  streams; DMA issue order = exactly the consumption order.
"""

import math
from contextlib import ExitStack

import numpy as np

import concourse.bass as bass
import concourse.mybir as mybir
import concourse.tile as tile
from concourse.bass_utils import run_bass_kernel_spmd
from concourse.vector_clock import ScopedClock

B, T, E = 2, 1024, 512
H, D = 8, 64
S = 128            # key stripe size
SC = 256           # query super-chunk size
NSC = T // SC      # 4
F32 = mybir.dt.float32
F32R = mybir.dt.float32r
F16 = mybir.dt.float16
EPS = 1e-6
NWARM = 8


def _install_drain_patch():
    """This walrus build rejects a Drain carrying >1 sem wait. Split the
    Tile-exit drain's waits across single-wait SP nops."""
    if getattr(tile.TileContext, "_drain_patch_installed", False):
        return

    def _patched(self, tick_clock, wait_clock):
        nc = self.nc
        pre = nc.sync.nop(nofuse=True)
        wait_clock.add_sem_waits(pre.ins, ScopedClock({None: tick_clock.global_clock}))
        waits = list(pre.ins.sync_info.on_wait or []) if pre.ins.sync_info else []
        if len(waits) > 1:
            pre.ins.sync_info.on_wait = waits[:1]
            for w in waits[1:]:
                n = nc.sync.nop(nofuse=True)
                if n.ins.sync_info is None:
                    n.ins.sync_info = mybir.SyncInfo(on_wait=[w], on_update=[])
                else:
                    n.ins.sync_info.on_wait = [w]
        nc.sync.drain()
        nc.all_engine_barrier()
        popped = nc._tile_sem_poison_stack.pop()
        assert popped is self._sem_poison

    tile.TileContext._drain_and_barrier = _patched
    tile.TileContext._drain_patch_installed = True


def _split_multi_waits(nc):
    """This walrus build only codegens ONE sync-wait command per instruction.
    Move excess waits onto same-engine NoOps inserted just before."""
    ctr = [0]

    def _mk_nop(engine, wait):
        ctr[0] += 1
        return mybir.InstNoOp(
            name=f"I-waitnop{ctr[0]}",
            engine=engine,
            ins=[],
            outs=[],
            sync_info=mybir.SyncInfo(on_wait=[wait], on_update=[]),
        )

    for f in nc.m.functions:
        for bb in f.blocks:
            new_insts = []
            for inst in bb.instructions:
                si = inst.sync_info
                waits = list(si.on_wait) if si and si.on_wait else []
                if len(waits) > 1:
                    for w in waits[:-1]:
                        new_insts.append(_mk_nop(inst.engine, w))
                    si.on_wait = waits[-1:]
                new_insts.append(inst)
            bb.instructions[:] = new_insts


def build_program() -> bass.Bass:
    _install_drain_patch()
    nc = bass.Bass()

    # wqkf: duplicated weight cols [qf_h0 | qf_h1 | kf_h0 | kf_h1], each 128 wide
    xt = nc.declare_dram_parameter("xt", [E, T], F16, isOutput=False)        # x[b].T
    wqkf = nc.declare_dram_parameter("wqkf", [E, 512], F16, isOutput=False)
    wvt = nc.declare_dram_parameter("wvt", [E, 128], F16, isOutput=False)    # [v0 v1].T
    ball = nc.declare_dram_parameter("ball", [640], F32, isOutput=False)     # dup'd qk biases + v bias
    csrep = nc.declare_dram_parameter("csrep", [128, T], F16, isOutput=False)  # [cos;sin]
    w2 = nc.declare_dram_parameter("w2", [128, E], F16, isOutput=False)
    identin = nc.declare_dram_parameter("identin", [128, 128], F16, isOutput=False)
    m0in = nc.declare_dram_parameter("m0in", [S, SC + S], F16, isOutput=False)  # [tri|ones|tri]
    out = nc.declare_dram_parameter("out", [T, E], F16, isOutput=True)
    out_r = out.rearrange("(s ci p) e -> p s ci e", ci=2, p=128)

    with tile.TileContext(nc) as tc, ExitStack() as ctx:
        singles = ctx.enter_context(tc.tile_pool(name="singles", bufs=1))
        kf_pool = ctx.enter_context(tc.tile_pool(name="kf", bufs=4))
        atm_pool = ctx.enter_context(tc.tile_pool(name="atm", bufs=3))
        osb_pool = ctx.enter_context(tc.tile_pool(name="osb", bufs=2))
        nrm_pool = ctx.enter_context(tc.tile_pool(name="nrm", bufs=4))
        ctx_pool = ctx.enter_context(tc.tile_pool(name="ctxp", bufs=2))
        pp_big = ctx.enter_context(tc.tile_pool(name="pp_big", bufs=2, space="PSUM"))
        pp_mm = ctx.enter_context(tc.tile_pool(name="pp_mm", bufs=2, space="PSUM"))
        pp_kt = ctx.enter_context(tc.tile_pool(name="pp_kt", bufs=1, space="PSUM"))
        pp_cs = ctx.enter_context(tc.tile_pool(name="pp_cs", bufs=3, space="PSUM"))

        # ---- input DMAs, exactly in consumption order on the sync queue ---
        ident = singles.tile([128, 128], F16)
        nc.sync.dma_start(out=ident, in_=identin[:, :])
        xt_s = singles.tile([128, 4, T], F16)
        xt_r = xt.rearrange("(kk p) t -> p kk t", p=128)
        wqkf_s = singles.tile([128, 4, 512], F16)
        wqkf_r = wqkf.rearrange("(kk p) c -> p kk c", p=128)
        biasall = singles.tile([128, 5], F32, name="biasall")
        cs_s = singles.tile([128, T], F16)
        for kk in range(4):
            nc.sync.dma_start(out=wqkf_s[:, kk, :], in_=wqkf_r[:, kk, :])
            nc.sync.dma_start(out=xt_s[:, kk, :], in_=xt_r[:, kk, :])
            if kk == 0:
                nc.sync.dma_start(out=biasall, in_=ball.rearrange("(g p) -> p g", p=128))
                nc.sync.dma_start(out=cs_s, in_=csrep[:, :])
        wvt_s = singles.tile([128, 4, 128], F16)
        nc.sync.dma_start(out=wvt_s, in_=wvt.rearrange("(kk p) c -> p kk c", p=128))
        m0_s = singles.tile([S, SC + S], F16)
        nc.sync.dma_start(out=m0_s, in_=m0in[:, :])
        w2_s = singles.tile([128, E], F16, name="w2_s")
        nc.sync.dma_start(out=w2_s, in_=w2[:, :])

        ones16 = singles.tile([1, 64], F16, name="ones16")
        nc.vector.memset(ones16, 1.0)
        onesz_col = singles.tile([128, 2], F16, name="onesz_col")
        nc.vector.memset(onesz_col[:, 0:1], 1.0)
        nc.vector.memset(onesz_col[:, 1:2], 0.0)

        # ---- PE warm-up on a zero scratch tile: starts as soon as the
        # vector engine is up, needs NO input DMA, keeps the HAM clock-gate
        # busy so real matmuls run at 2.4 GHz.
        junk = singles.tile([128, 512], F16, name="junk")
        nc.vector.memset(junk, 0.0)
        for wi in range(NWARM):
            psw = pp_big.tile([128, 512], F32, tag="big", name=f"psw{wi}")
            nc.tensor.matmul(psw, junk[:, 0:128], junk, start=True, stop=True)

        # per-head stacked feature tiles [cos;sin] x t
        qfT = [singles.tile([128, T], F16, name=f"qfT{h}") for h in range(2)]
        kfT = [singles.tile([128, T], F16, name=f"kfT{h}") for h in range(2)]
        vT = singles.tile([128, T], F16, name="vT")
        state = [singles.tile([128, D + 2], F16, name=f"state{h}") for h in range(2)]
        # persistent V' ring: [head][stripe], ones/pad cols written once
        vp_ring = [[singles.tile([S, D + 2], F16, name=f"vpr{h}_{ci}")
                    for ci in range(2)] for h in range(2)]
        for h in range(2):
            for ci in range(2):
                nc.scalar.copy(vp_ring[h][ci][:, D:D + 2], onesz_col)

        # ---- q/k/v features: block-outer, kk-inner, t-halves paired ------
        # block bi: 0=qf_h0, 1=qf_h1, 2=kf_h0, 3=kf_h1, 4=v
        for bi in range(5):
            ps_h = [pp_big.tile([128, 512], F32, tag="big", name=f"psB{bi}_{th}")
                    for th in range(2)]
            for kk in range(4):
                w = wqkf_s[:, kk, bi * 128:(bi + 1) * 128] if bi < 4 else wvt_s[:, kk, :]
                for th in range(2):
                    nc.tensor.matmul(ps_h[th], w, xt_s[:, kk, th * 512:(th + 1) * 512],
                                     start=(kk == 0), stop=(kk == 3))
            for th in range(2):
                tslh = slice(th * 512, (th + 1) * 512)
                if bi < 4:
                    dst = [qfT[0], qfT[1], kfT[0], kfT[1]][bi]
                    nc.scalar.activation(
                        out=dst[:, tslh],
                        in_=ps_h[th],
                        func=mybir.ActivationFunctionType.Relu,
                        bias=biasall[:, bi:bi + 1],
                        scale=1.0,
                    )
                    nc.gpsimd.tensor_mul(dst[:, tslh], dst[:, tslh], cs_s[:, tslh])
                else:
                    nc.scalar.activation(
                        out=vT[:, tslh],
                        in_=ps_h[th],
                        func=mybir.ActivationFunctionType.Identity,
                        bias=biasall[:, 4:5],
                        scale=1.0,
                    )

        # ---- attention, 256-wide query super-chunks, software-pipelined:
        # super-chunk sc's matmuls are emitted BEFORE sc-1's norm/out-proj
        # tail so the PE queue never stalls on the reciprocal chain.
        def emit_tail(psc2, sc):
            # reciprocal of the [1,512] norm row is ~6.5ns/elem on one DVE
            # lane; bounce it through a [128,4] partition-major layout via two
            # small SBUF DMAs so the reciprocal runs on 128 lanes (~0.2us).
            nrow = nrm_pool.tile([1, 2 * SC], F32, tag="nrow", name=f"nrow{sc}")
            nc.scalar.copy(nrow, psc2[D:D + 1, :, :])
            nP = nrm_pool.tile([128, 4], F32, tag="nP", name=f"nP{sc}")
            nc.gpsimd.dma_start(out=nP, in_=nrow)
            nPr = nrm_pool.tile([128, 4], F32, tag="nPr", name=f"nPr{sc}")
            nc.vector.reciprocal(nPr, nP)
            rcpf = nrm_pool.tile([1, 2 * SC], F32, tag="rcpf", name=f"rcpf{sc}")
            nc.gpsimd.dma_start(out=rcpf, in_=nPr)
            rcp16 = nrm_pool.tile([1, 2 * SC], F16, tag="rcp16", name=f"rcp16_{sc}")
            nc.gpsimd.tensor_copy(rcp16, rcpf)
            ps_rcp = pp_mm.tile([D, 2 * SC], F32, tag="mm", name=f"psrcp{sc}")
            nc.tensor.matmul(ps_rcp, ones16, rcp16, start=True, stop=True)
            rcp2 = nrm_pool.tile([D, 2 * SC], F16, tag="rcp2", name=f"rcp2{sc}")
            nc.scalar.copy(rcp2[:, 0:SC], ps_rcp[:, 0:SC])
            nc.vector.tensor_copy(rcp2[:, SC:2 * SC], ps_rcp[:, SC:2 * SC])

            # normalized two-head context block [2d, t], then one combined
            # out-projection per 128 queries (full 128 contraction)
            ctx2 = ctx_pool.tile([128, SC], F16, tag="ctx2", name=f"ctx2{sc}")
            nc.vector.tensor_mul(ctx2[0:D, :], psc2[0:D, 0, :], rcp2[:, 0:SC])
            nc.vector.tensor_mul(ctx2[D:2 * D, :], psc2[0:D, 1, :], rcp2[:, SC:2 * SC])

            o_s2 = osb_pool.tile([128, 2, E], F16, tag="osb", name=f"os{sc}")
            for ci in range(2):
                ps_o = pp_big.tile([128, E], F32, tag="big", name=f"pso{sc}_{ci}")
                nc.tensor.matmul(ps_o, ctx2[:, ci * S:(ci + 1) * S], w2_s,
                                 start=True, stop=True)
                if ci == 0:
                    nc.vector.tensor_copy(o_s2[:, ci, :], ps_o)
                else:
                    nc.scalar.copy(o_s2[:, ci, :], ps_o)
            nc.sync.dma_start(out=out_r[:, sc, :, :], in_=o_s2)

        pipe = []  # (psc2, sc) tuples awaiting their norm/out-proj tails
        for sc in range(NSC):
            t0 = sc * SC
            band = slice(t0, t0 + SC)
            sub = [slice(t0, t0 + S), slice(t0 + S, t0 + 2 * S)]

            # stripe transposes: kfT/vT [*, t] -> [t, *] per 128-stripe
            kfeat = [[None, None], [None, None]]  # [ci][h]
            vp = [[None, None], [None, None]]     # [ci][h]
            for ci in range(2):
                ps_kt = pp_kt.tile([128, 384], F16, tag="kt", name=f"pskt{sc}_{ci}")
                for h in range(2):
                    kfeat[ci][h] = kf_pool.tile(
                        [S, 128], F16, tag=f"kf{h}", name=f"kfeat{sc}_{ci}_{h}")
                    nc.tensor.transpose(
                        ps_kt[:, h * 128:(h + 1) * 128], kfT[h][:, sub[ci]], ident)
                nc.vector.tensor_copy(kfeat[ci][0], ps_kt[:, 0:128])
                nc.scalar.copy(kfeat[ci][1], ps_kt[:, 128:256])
                nc.tensor.transpose(ps_kt[:, 256:384], vT[:, sub[ci]], ident)
                for h in range(2):
                    vp[ci][h] = vp_ring[h][ci]
                nc.vector.tensor_copy(vp[ci][0][:, 0:D], ps_kt[:, 256:256 + D])
                nc.scalar.copy(vp[ci][1][:, 0:D], ps_kt[:, 256 + D:256 + 2 * D])

            # both heads' ctx^T (+norm row 64) share one PSUM bank so two
            # super-chunks can be in flight (pipelined tail)
            psc2 = pp_cs.tile([D + 2, 2, SC], F32, tag="cs", name=f"psc{sc}")
            for h in range(2):
                # stripe 0 scores the whole band; stripe 1 only its own half
                # (both score blocks share one bank: one accumulation group)
                psa = pp_mm.tile([S, SC + S], F32, tag="mm", name=f"psa{sc}_{h}")
                nc.tensor.matmul(psa[:, 0:SC], kfT[h][:, sub[0]], qfT[h][:, band],
                                 start=True, stop=False)
                nc.tensor.matmul(psa[:, SC:SC + S], kfT[h][:, sub[1]], qfT[h][:, sub[1]],
                                 start=False, stop=True)
                atm = atm_pool.tile([S, SC + S], F16, tag="atm", name=f"atm{sc}_{h}")
                nc.vector.tensor_mul(atm, psa, m0_s)
                atm0, atm1 = atm[:, 0:SC], atm[:, SC:SC + S]

                # ctx^T: prefix-state inter + two stripe intras; the two heads
                # form ONE accumulation group (h0 starts/clears the bank, h1's
                # fresh-region writes land via has_written=0)
                st = (h == 0)
                if sc > 0:
                    nc.tensor.matmul(psc2[:, h, :], state[h], qfT[h][:, band],
                                     start=st, stop=False)
                    nc.tensor.matmul(psc2[:, h, :], vp[0][h], atm0,
                                     start=False, stop=False)
                    nc.tensor.matmul(psc2[:, h, S:SC], vp[1][h], atm1,
                                     start=False, stop=(h == 1))
                else:
                    nc.tensor.matmul(psc2[:, h, :], vp[0][h], atm0,
                                     start=st, stop=False)
                    nc.tensor.matmul(psc2[:, h, S:SC], vp[1][h], atm1,
                                     start=False, stop=(h == 1))

                # state += Kf^T V' over both stripes
                ps_s = pp_mm.tile([128, D + 2], F32, tag="mm", name=f"pss{sc}_{h}")
                nc.tensor.matmul(ps_s, kfeat[0][h], vp[0][h], start=True, stop=False)
                nc.tensor.matmul(ps_s, kfeat[1][h], vp[1][h], start=False, stop=True)
                if sc == 0:
                    nc.vector.tensor_copy(state[h], ps_s)
                else:
                    nc.vector.tensor_add(state[h], state[h], ps_s)

            pipe.append((psc2, sc))
            if len(pipe) > 2:
                emit_tail(*pipe.pop(0))
        for p in pipe:
            emit_tail(*p)

    _split_multi_waits(nc)
    return nc


_PROGRAM = None


def _get_program():
    global _PROGRAM
    if _PROGRAM is None:
        _PROGRAM = build_program()
    return _PROGRAM


def _make_in_maps(x, w_qkv, b_qkv, w_out):
    pos = np.arange(T, dtype=np.float32)
    ang = (math.pi / 2) * pos / T
    cosw = np.cos(ang).astype(np.float32)
    sinw = np.sin(ang).astype(np.float32)
    csrep = np.concatenate([
        np.broadcast_to(cosw[None, :], (D, T)),
        np.broadcast_to(sinw[None, :], (D, T)),
    ], 0).astype(np.float16)
    tri = np.triu(np.ones((S, S), np.float16))
    m0 = np.concatenate([tri, np.ones((S, S), np.float16), tri], 1)

    in_maps = []
    for i in range(8):
        b, g = divmod(i, 4)
        h0, h1 = 2 * g, 2 * g + 1
        wq = lambda h: w_qkv[h * D:(h + 1) * D]
        wk = lambda h: w_qkv[E + h * D:E + (h + 1) * D]
        wv = lambda h: w_qkv[2 * E + h * D:2 * E + (h + 1) * D]
        bq = lambda h: b_qkv[h * D:(h + 1) * D]
        bk = lambda h: b_qkv[E + h * D:E + (h + 1) * D]
        bv = lambda h: b_qkv[2 * E + h * D:2 * E + (h + 1) * D]
        hcols = np.r_[h0 * D:(h0 + 1) * D, h1 * D:(h1 + 1) * D]
        wqkf = np.concatenate([
            wq(h0), wq(h0), wq(h1), wq(h1), wk(h0), wk(h0), wk(h1), wk(h1)
        ], 0).T
        ball = np.concatenate([
            bq(h0), bq(h0), bq(h1), bq(h1), bk(h0), bk(h0), bk(h1), bk(h1),
            bv(h0), bv(h1)
        ]).astype(np.float32)
        in_maps.append({
            "xt": np.ascontiguousarray(x[b].T).astype(np.float16),
            "wqkf": np.ascontiguousarray(wqkf).astype(np.float16),
            "wvt": np.ascontiguousarray(
                np.concatenate([wv(h0), wv(h1)], 0).T).astype(np.float16),
            "ball": np.ascontiguousarray(ball),
            "csrep": csrep,
            "w2": np.ascontiguousarray(w_out[:, hcols].T).astype(np.float16),
            "identin": np.eye(128, dtype=np.float16),
            "m0in": m0,
        })
    return in_maps


def run(inputs, trace=False):
    x = np.asarray(inputs["x"], dtype=np.float32)
    w_qkv = np.asarray(inputs["w_qkv"], dtype=np.float32)
    b_qkv = np.asarray(inputs["b_qkv"], dtype=np.float32)
    w_out = np.asarray(inputs["w_out"], dtype=np.float32)
    b_out = np.asarray(inputs["b_out"], dtype=np.float32)

    nc = _get_program()
    in_maps = _make_in_maps(x, w_qkv, b_qkv, w_out)
    res = run_bass_kernel_spmd(nc, in_maps, list(range(8)), trace=trace)

    out = np.empty((B, T, E), dtype=np.float32)
    for b in range(B):
        acc = res.results[4 * b]["out"].astype(np.float32)
        for g in range(1, 4):
            acc = acc + res.results[4 * b + g]["out"].astype(np.float32)
        out[b] = acc + b_out[None, :]
    return out, res


def kernel(**inputs) -> np.ndarray:
    out, _ = run(inputs, trace=False)
    return out
